# revision 2
# baseline (speedup 1.0000x reference)
"""Trainium2 Bass kernel for nn_Actions_block_14388140442036 (gnn_message_passing).

The reference network is entirely linear (no activations), so the output
    out = segment_sum(actions) @ pol_W + pol_b
collapses to per-effect scalars:
    p[j] = actions[j] @ pol_W  (a dot product against fused weight vectors)
followed by a scalar segment-sum.  Folding pol_W through each branch:

  glob branch:  p_g[i] = (globs @ w1)[U[i]]     + action_globs[i]. w2 + cg
  node branch:  p_n[i] = (nodes @ w3)[V[i]]     + action_nodes[i]. w4 + cn
  edge branch:  p_e[i] = (edges @ u1)[E[i]] + (nodes @ wr)[row[E[i]]]
                        + (nodes @ wc)[col[E[i]]] + action_edges[i]. wv + ce

where  w1|w2 = glob_W @ pol_W,  w3|w4 = node_W @ pol_W,
       u1|u2 = e2_W @ pol_W,    wr|wv|wc = e1_W @ u2.

The memory-heavy work -- streaming edges (205MB), nodes (51MB, x3 weight
vectors) and the action features (19MB) through per-row dot products -- runs
on 8 NeuronCores (row-sharded, replicated weight tiles).  The host does the
tiny fused-weight precompute, the scalar gathers and the segment sum.
"""

import numpy as np

import concourse.bacc as bacc
import concourse.mybir as mybir
import concourse.tile as tile
from concourse.bass_utils import run_bass_kernel_spmd

# ---- problem constants (hardcoded; kernel.py must be self-contained) ----
HID = 128
FEAT = 16
N_NODES = 100000
N_EDGES = 400000
N_PER = 100000
A_TOTAL = 300000
NUM_ACTIONS = 75000
N_CORES = 8

E_SH = N_EDGES // N_CORES   # 50000 edge rows per core
N_SH = N_NODES // N_CORES   # 12500 node rows per core
A_SH = N_PER // N_CORES     # 12500 action-effect rows per core

# Row->SBUF packing: C consecutive rows live in one partition's free dim, so a
# [128, C*width] tile covers 128*C rows with C*width*4B contiguous descriptors.
C_E = 6    # edges:  tile [128, 768],  768 rows, 3KB/descriptor
T_E = 65   # 65*768 = 49920 main rows, tail 80
E_TAIL = E_SH - T_E * 128 * C_E          # 80

C_N = 4    # nodes:  tile [128, 512],  512 rows
T_N = 24   # 24*512 = 12288 main rows, tail 212 rows at C=2 -> [106, 256]
N_TAIL = N_SH - T_N * 128 * C_N          # 212

C_A = 32   # apack:  tile [128, 1536], 4096 rows (48 floats/row)
T_A = 3    # 3*4096 = 12288 main rows, tail 212 rows at C=2 -> [106, 96]
A_TAIL = A_SH - T_A * 128 * C_A          # 212

# weight-tile column layout inside the packed "wts" input [128, 3840]
W_U1 = (0, 768)        # u1 tiled x6
W_W3 = (768, 1280)     # w3 tiled x4
W_WR = (1280, 1792)    # wr tiled x4
W_WC = (1792, 2304)    # wc tiled x4
W_A48 = (2304, 3840)   # [w2|w4|wv] (48) tiled x32
WTS_COLS = 3840

F32 = mybir.dt.float32
AX = mybir.AxisListType.X

_CACHE = {}


def _build_program():
    nc = bacc.Bacc("TRN2", target_bir_lowering=False, debug=False,
                   num_devices=N_CORES)

    edges_in = nc.dram_tensor("edges_in", [E_SH, HID], F32, kind="ExternalInput").ap()
    nodes_in = nc.dram_tensor("nodes_in", [N_SH, HID], F32, kind="ExternalInput").ap()
    apack_in = nc.dram_tensor("apack_in", [A_SH, 3 * FEAT], F32, kind="ExternalInput").ap()
    wts_in = nc.dram_tensor("wts_in", [128, WTS_COLS], F32, kind="ExternalInput").ap()

    qe_out = nc.dram_tensor("qe_out", [128, T_E * C_E + 1], F32, kind="ExternalOutput").ap()
    qn_out = nc.dram_tensor("qn_out", [128, 300], F32, kind="ExternalOutput").ap()
    pa_out = nc.dram_tensor("pa_out", [128, T_A * C_A * 3 + 6], F32, kind="ExternalOutput").ap()

    with tile.TileContext(nc) as tc:
        with (
            tc.tile_pool(name="wpool", bufs=1) as wpool,
            tc.tile_pool(name="dpool", bufs=4) as dpool,
            tc.tile_pool(name="tpool", bufs=4) as tpool,
            tc.tile_pool(name="accpool", bufs=1) as accpool,
        ):
            wt = wpool.tile([128, WTS_COLS], F32)
            nc.sync.dma_start(wt[:], wts_in[:])

            qe_acc = accpool.tile([128, T_E * C_E + 1], F32)
            qn_acc = accpool.tile([128, 300], F32)
            pa_acc = accpool.tile([128, T_A * C_A * 3 + 6], F32)

            # ---------------- edges: qe = edges . u1 ----------------
            e_main = edges_in[0:T_E * 128 * C_E, :].rearrange(
                "(t p c) f -> t p (c f)", p=128, c=C_E)
            u1b = wt[:, W_U1[0]:W_U1[1]]
            for t in range(T_E):
                d = dpool.tile([128, C_E * HID], F32, tag="ed")
                nc.sync.dma_start(d[:], e_main[t])
                tmp = tpool.tile([128, C_E * HID], F32, tag="et")
                nc.vector.tensor_mul(tmp[:], d[:], u1b)
                nc.vector.reduce_sum(
                    qe_acc[:, t * C_E:(t + 1) * C_E],
                    tmp[:].rearrange("p (c f) -> p c f", f=HID), axis=AX)
            # tail: 80 rows, one per partition
            d = dpool.tile([128, HID], F32, tag="ed")
            nc.sync.dma_start(d[:E_TAIL, :], edges_in[T_E * 128 * C_E:E_SH, :])
            tmp = tpool.tile([128, HID], F32, tag="et")
            nc.vector.tensor_mul(tmp[:E_TAIL, :], d[:E_TAIL, :], u1b[:E_TAIL, :HID])
            nc.vector.reduce_sum(qe_acc[:E_TAIL, T_E * C_E:T_E * C_E + 1],
                                 tmp[:E_TAIL, :], axis=AX)

            # ------------- nodes: qn/qr/qc = nodes . {w3,wr,wc} -------------
            n_main = nodes_in[0:T_N * 128 * C_N, :].rearrange(
                "(t p c) f -> t p (c f)", p=128, c=C_N)
            wslices = [wt[:, W_W3[0]:W_W3[1]], wt[:, W_WR[0]:W_WR[1]],
                       wt[:, W_WC[0]:W_WC[1]]]
            for t in range(T_N):
                d = dpool.tile([128, C_N * HID], F32, tag="nd")
                nc.sync.dma_start(d[:], n_main[t])
                for w in range(3):
                    tmp = tpool.tile([128, C_N * HID], F32, tag="nt")
                    nc.vector.tensor_mul(tmp[:], d[:], wslices[w])
                    nc.vector.reduce_sum(
                        qn_acc[:, w * 100 + t * C_N: w * 100 + (t + 1) * C_N],
                        tmp[:].rearrange("p (c f) -> p c f", f=HID), axis=AX)
            # tail: 212 rows at C=2 -> [106, 256]
            n_tail = nodes_in[T_N * 128 * C_N:N_SH, :].rearrange(
                "(p c) f -> p (c f)", c=2)
            NP_T = N_TAIL // 2  # 106 partitions
            d = dpool.tile([128, 2 * HID], F32, tag="nd")
            nc.sync.dma_start(d[:NP_T, :], n_tail)
            for w in range(3):
                tmp = tpool.tile([128, 2 * HID], F32, tag="nt")
                nc.vector.tensor_mul(tmp[:NP_T, :], d[:NP_T, :], wslices[w][:NP_T, :2 * HID])
                nc.vector.reduce_sum(
                    qn_acc[:NP_T, w * 100 + T_N * C_N: w * 100 + T_N * C_N + 2],
                    tmp[:NP_T, :].rearrange("p (c f) -> p c f", f=HID), axis=AX)

            # ---- action features: pa = [ag|an|ae] . [w2|w4|wv] per 16 ----
            a_main = apack_in[0:T_A * 128 * C_A, :].rearrange(
                "(t p c) f -> t p (c f)", p=128, c=C_A)
            a48b = wt[:, W_A48[0]:W_A48[1]]
            for t in range(T_A):
                d = dpool.tile([128, C_A * 48], F32, tag="ad")
                nc.sync.dma_start(d[:], a_main[t])
                tmp = tpool.tile([128, C_A * 48], F32, tag="at")
                nc.vector.tensor_mul(tmp[:], d[:], a48b)
                nc.vector.reduce_sum(
                    pa_acc[:, t * C_A * 3:(t + 1) * C_A * 3],
                    tmp[:].rearrange("p (s f) -> p s f", f=FEAT), axis=AX)
            # tail: 212 rows at C=2 -> [106, 96]
            a_tail = apack_in[T_A * 128 * C_A:A_SH, :].rearrange(
                "(p c) f -> p (c f)", c=2)
            AP_T = A_TAIL // 2  # 106
            d = dpool.tile([128, 96], F32, tag="ad")
            nc.sync.dma_start(d[:AP_T, :], a_tail)
            tmp = tpool.tile([128, 96], F32, tag="at")
            nc.vector.tensor_mul(tmp[:AP_T, :], d[:AP_T, :], a48b[:AP_T, :96])
            nc.vector.reduce_sum(
                pa_acc[:AP_T, T_A * C_A * 3:T_A * C_A * 3 + 6],
                tmp[:AP_T, :].rearrange("p (s f) -> p s f", f=FEAT), axis=AX)

            nc.sync.dma_start(qe_out[:], qe_acc[:])
            nc.sync.dma_start(qn_out[:], qn_acc[:])
            nc.sync.dma_start(pa_out[:], pa_acc[:])

    nc.compile()
    return nc


def _get_program():
    if "nc" not in _CACHE:
        _CACHE["nc"] = _build_program()
    return _CACHE["nc"]


def _unscramble_qe(arr):
    """[128, 391] -> [50000] in original row order."""
    main = arr[:, :T_E * C_E].reshape(128, T_E, C_E).transpose(1, 0, 2).reshape(-1)
    tail = arr[:E_TAIL, T_E * C_E]
    return np.concatenate([main, tail])


def _unscramble_qn(arr):
    """[128, 300] -> 3 x [12500] (w3, wr, wc dots) in original row order."""
    out = []
    for w in range(3):
        main = arr[:, w * 100: w * 100 + T_N * C_N].reshape(
            128, T_N, C_N).transpose(1, 0, 2).reshape(-1)
        tail = arr[:N_TAIL // 2, w * 100 + T_N * C_N: w * 100 + T_N * C_N + 2].reshape(-1)
        out.append(np.concatenate([main, tail]))
    return out


def _unscramble_pa(arr):
    """[128, 294] -> [12500, 3] (ag.w2, an.w4, ae.wv) in original row order."""
    main = arr[:, :T_A * C_A * 3].reshape(128, T_A, C_A, 3).transpose(1, 0, 2, 3)
    main = main.reshape(-1, 3)
    tail = arr[:A_TAIL // 2, T_A * C_A * 3:].reshape(-1, 3)
    return np.concatenate([main, tail], axis=0)


def kernel(**inputs):
    globs = inputs["globs"]
    nodes = np.ascontiguousarray(inputs["nodes"])
    edges = np.ascontiguousarray(inputs["edges"])
    action_globs = inputs["action_globs"]
    action_nodes = inputs["action_nodes"]
    action_edges = inputs["action_edges"]
    glob_W = inputs["glob_W"]; glob_b = inputs["glob_b"]
    node_W = inputs["node_W"]; node_b = inputs["node_b"]
    e1_W = inputs["e1_W"]; e1_b = inputs["e1_b"]
    e2_W = inputs["e2_W"]; e2_b = inputs["e2_b"]
    pol_W = inputs["pol_W"]; pol_b = inputs["pol_b"]
    row = inputs["row"]; col = inputs["col"]
    U = inputs["U"]; UA = inputs["UA"]; V = inputs["V"]; VA = inputs["VA"]
    E = inputs["E"]; EA = inputs["EA"]
    actions_batch = inputs["actions_batch"]

    # ---- fused weight vectors (float64 for accuracy; cast to f32 on device) ----
    polW = pol_W.astype(np.float64)[:, 0]                 # [128]
    g_f = glob_W.astype(np.float64) @ polW                # [144]
    n_f = node_W.astype(np.float64) @ polW                # [144]
    e2_f = e2_W.astype(np.float64) @ polW                 # [256]
    u1, u2 = e2_f[:HID], e2_f[HID:]
    e1_f = e1_W.astype(np.float64) @ u2                   # [272]
    w1, w2 = g_f[:HID], g_f[HID:]
    w3, w4 = n_f[:HID], n_f[HID:]
    wr, wv, wc = e1_f[:HID], e1_f[HID:HID + FEAT], e1_f[HID + FEAT:]
    cg = float(glob_b.astype(np.float64) @ polW)
    cn = float(node_b.astype(np.float64) @ polW)
    ce = float(e2_b.astype(np.float64) @ polW + e1_b.astype(np.float64) @ u2)

    # packed replicated weight tiles
    wts = np.empty((128, WTS_COLS), np.float32)
    wts[:, W_U1[0]:W_U1[1]] = np.tile(u1.astype(np.float32), (128, C_E))
    wts[:, W_W3[0]:W_W3[1]] = np.tile(w3.astype(np.float32), (128, C_N))
    wts[:, W_WR[0]:W_WR[1]] = np.tile(wr.astype(np.float32), (128, C_N))
    wts[:, W_WC[0]:W_WC[1]] = np.tile(wc.astype(np.float32), (128, C_N))
    w48 = np.concatenate([w2, w4, wv]).astype(np.float32)
    wts[:, W_A48[0]:W_A48[1]] = np.tile(w48, (128, C_A))

    # packed action features [N_PER, 48] = [ag | an | ae]
    apack = np.empty((N_PER, 3 * FEAT), np.float32)
    apack[:, :FEAT] = action_globs
    apack[:, FEAT:2 * FEAT] = action_nodes
    apack[:, 2 * FEAT:] = action_edges

    nc = _get_program()
    in_maps = []
    for c in range(N_CORES):
        in_maps.append({
            "edges_in": edges[c * E_SH:(c + 1) * E_SH],
            "nodes_in": nodes[c * N_SH:(c + 1) * N_SH],
            "apack_in": apack[c * A_SH:(c + 1) * A_SH],
            "wts_in": wts,
        })
    res = run_bass_kernel_spmd(nc, in_maps, core_ids=list(range(N_CORES)))

    qe = np.empty(N_EDGES, np.float64)
    qn = np.empty(N_NODES, np.float64)
    qr = np.empty(N_NODES, np.float64)
    qc = np.empty(N_NODES, np.float64)
    pa = np.empty((N_PER, 3), np.float64)
    for c in range(N_CORES):
        r = res.results[c]
        qe[c * E_SH:(c + 1) * E_SH] = _unscramble_qe(r["qe_out"])
        qn_c, qr_c, qc_c = _unscramble_qn(r["qn_out"])
        qn[c * N_SH:(c + 1) * N_SH] = qn_c
        qr[c * N_SH:(c + 1) * N_SH] = qr_c
        qc[c * N_SH:(c + 1) * N_SH] = qc_c
        pa[c * A_SH:(c + 1) * A_SH] = _unscramble_pa(r["pa_out"])

    # ---- host: gathers, scatter into action slots, segment sum ----
    qg = globs.astype(np.float64) @ w1                    # [512]
    p_g = qg[U] + pa[:, 0] + cg
    p_n = qn[V] + pa[:, 1] + cn
    p_e = qe[E] + qr[row[E]] + qc[col[E]] + pa[:, 2] + ce

    actions_p = np.zeros(A_TOTAL, np.float64)
    actions_p[UA] = p_g
    actions_p[VA] = p_n
    actions_p[EA] = p_e

    # torch-style _norm: consecutive group ids starting at actions_batch[0]
    ab = actions_batch.astype(np.int64)
    changed = ab[1:] != ab[:-1]
    seg = int(ab[0]) + np.concatenate([[0], np.cumsum(changed)])
    if seg[0] >= 0 and seg[-1] < NUM_ACTIONS:
        agg = np.bincount(seg, weights=actions_p, minlength=NUM_ACTIONS)[:NUM_ACTIONS]
    else:  # jax segment_sum drops out-of-range ids
        agg = np.zeros(NUM_ACTIONS, np.float64)
        valid = (seg >= 0) & (seg < NUM_ACTIONS)
        np.add.at(agg, seg[valid], actions_p[valid])

    out = agg + float(pol_b.astype(np.float64)[0])
    return out.astype(np.float32)[:, None]
